# revision 16
# baseline (speedup 1.0000x reference)
"""DiscreteBipartiteFlow forward on 8 Trainium2 NeuronCores.

Math (forward pass only):
  masked = mask * inputs                      (mask = 1 at odd l, 0 at even l)
  h   = relu(masked.reshape(B, L*V) @ W1 + b1)
  net = (h @ W2 + b2).reshape(B, L, 2V)
  loc, scale = argmax one-hots of net[..., :V], net[..., V:]
  out[odd l]  = inputs
  out[even l] = onehot((inv(scale) * ((tok - loc) mod V)) mod V), or 0 if scale==0

Sharding (8 cores): mm1 tensor-parallel over hidden; all-gather h; mm2
tensor-parallel over output columns (only the 736 even-position columns).

Precision: fp16 main passes + ONE DoubleRow dual-fp8 correction pass
(identical numerics to the validated baseline: 4 argmax flips, rel_err
0.0074, gate 2e-2 ~ 26 flips).

Schedule (v2): the runtime comm BARRIER (~50us) runs concurrently with
the input DMAs + mm1; the h AllGathers are the first collectives (no
warm-up dummy).  W2 streams through the SAME rotating weight buffers
mm1's weights used (tag-shared tile pool, 5 bufs each), issued on the
scalar HWDGE ring ahead of all AG-gated traffic, so ~7 of 8 blocks land
before the gather window opens.  The gathered-h unpack rides the sync
ring (idle after the input loads).  w2 fp8 hi slots are derived on-chip
from the fp16 blocks (saves 3MB of hot-window HBM).  mm2 runs per
column-half; the last block group and all of sweep 2 run batch-tile-
major with per-(half, batch-tile) epilogues so argmax/flow hide under
the remaining matmuls; the flow inverse uses a one-hot LUT reduce
instead of a serial mod-exponentiation chain.
"""

import numpy as np
import ml_dtypes

B, L, V = 512, 256, 23
H = 4096
NCORES = 8
HS = H // NCORES          # 512  hidden shard
HM = HS // 128            # 4    local hidden tiles
PS = L // NCORES          # 32   positions per core
EP = PS // 2              # 16   even positions per core
CW = PS * 2 * V           # 1472 net columns per core (incl. unused odd)
CE = EP * 2 * V           # 736  even-position net columns
NCH = 2                   # column-half sweeps for mm2
CC = CE // NCH            # 368  columns per sweep
KT1 = (L // 2) * V // 128  # 23 contraction tiles for mm1
KP1 = 12                  # mm1 pair tiles (2944 rows -> 1472 pairs, padded)
XG = 3                    # xt DMA groups of 8 k-tiles (8+8+7)
KT2 = H // 128            # 32 contraction tiles for mm2
MT = B // 128             # 4 batch tiles

SC_W = 2.0**13            # w_lo / w1_lo quantization scale
SC_H = 2.0**12            # h_lo quantization scale
SC_WH = 2.0               # w_hi quantization scale (12 + 1 = 13)
DESC = 2.0**-13           # corr descale

BIG = 64.0
MAGIC = 12582912.0        # 1.5 * 2^23: float32 round-to-int domain
FP16 = np.float16
E4M3 = ml_dtypes.float8_e4m3   # TRN FP8_EXP4: max +-240, matches in-range

_cache = {}


def _inv_table():
    return np.array([0] + [pow(a, -1, V) for a in range(1, V)],
                    dtype=np.float32)


def _build():
    import concourse.mybir as mybir
    import concourse.tile as tile
    from concourse import bacc

    fp32 = mybir.dt.float32
    fp16 = mybir.dt.float16
    fp8 = mybir.dt.float8e4
    u8 = mybir.dt.uint8
    Alu = mybir.AluOpType
    Act = mybir.ActivationFunctionType
    DR = mybir.MatmulPerfMode.DoubleRow
    AxX = mybir.AxisListType.X

    nc = bacc.Bacc("TRN2", target_bir_lowering=False, debug=False,
                   num_devices=NCORES)

    # ---- per-core inputs ----
    xtg = nc.dram_tensor("xtg", [XG, 128, 8 * B], fp16, kind="ExternalInput")
    xp8 = nc.dram_tensor("xp8", [128, KP1, 2, B], fp8, kind="ExternalInput")
    w1m = nc.dram_tensor("w1m", [HM, 128, KT1 * 128], fp16,
                         kind="ExternalInput")
    w1pm = nc.dram_tensor("w1pm", [HM, 128, KP1 * 256], fp8,
                          kind="ExternalInput")
    b1s = nc.dram_tensor("b1s", [128, HM], fp32, kind="ExternalInput")
    w216 = nc.dram_tensor("w216", [NCH, HM, 128, 8 * CC], fp16,
                          kind="ExternalInput")
    w2p8 = nc.dram_tensor("w2p8", [NCH, HM, 128, 8 * CC], fp8,
                          kind="ExternalInput")
    b2r = nc.dram_tensor("b2r", [128, CE], fp32, kind="ExternalInput")
    inpe = nc.dram_tensor("inpe", [128, MT * EP * V], fp32,
                          kind="ExternalInput")
    oute = nc.dram_tensor("oute", [128, MT * EP * V], fp32,
                          kind="ExternalOutput")

    # ---- constants: iota | BIG-iota | inv_table ----
    iota_np = np.arange(V, dtype=np.float32)[None, :].repeat(128, 0)
    cc_np = np.concatenate(
        [iota_np, BIG - iota_np, _inv_table()[None, :].repeat(128, 0)], axis=1)
    c_cc = nc.inline_tensor(np.ascontiguousarray(cc_np), name="c_cc")

    with tile.TileContext(nc) as tc:
        with (
            tc.tile_pool(name="persist", bufs=1) as persist,
            tc.tile_pool(name="wst", bufs=1) as wst,
            tc.tile_pool(name="hwork", bufs=2) as hwork,
            tc.tile_pool(name="ep", bufs=2) as ep,
            tc.tile_pool(name="small", bufs=1) as small,
            tc.tile_pool(name="ps", bufs=1, space="PSUM") as ps,
            tc.tile_pool(name="dram", bufs=1, space="DRAM") as dram,
        ):
            # ---------- dummy collective: absorbs first-collective setup --
            warm_in = dram.tile([1, 16], fp32, tag="warm_in")
            warm_out = dram.tile([NCORES, 16], fp32, tag="warm_out",
                                 addr_space="Shared")
            nc.gpsimd.collective_compute(
                "AllGather", Alu.bypass,
                replica_groups=[list(range(NCORES))],
                ins=[warm_in.opt()], outs=[warm_out.opt()],
            )

            # ---------- bulk loads: sync ring (mm1-critical first) --------
            cc_t = persist.tile([128, 3 * V], fp32, tag="ccst")
            nc.sync.dma_start(cc_t[:], c_cc[:])
            iota_t = cc_t[:, :V]
            cbi_t = cc_t[:, V:2 * V]
            invt_t = cc_t[:, 2 * V:]
            b1_t = persist.tile([128, HM], fp32, tag="b1")
            nc.sync.dma_start(b1_t[:], b1s[:])
            xt_t = []
            for g2 in range(XG):
                ng2 = 8 if g2 < XG - 1 else KT1 - 8 * (XG - 1)
                t = persist.tile([128, ng2, B], fp16, tag=f"xtg{g2}",
                                 name=f"xtg{g2}")
                nc.sync.dma_start(t[:], xtg[g2][:, :ng2 * B].rearrange(
                    "p (e b) -> p e b", b=B))
                xt_t.append(t)
            w1m_t = []
            for m in range(HM):
                tw = wst.tile([128, KT1, 128], fp16, tag="wblk", bufs=6,
                              name=f"w1m{m}")
                nc.sync.dma_start(tw[:], w1m[m].rearrange(
                    "p (k c) -> p k c", c=128))
                w1m_t.append(tw)
            inpe_t = persist.tile([128, MT, EP * V], fp32, tag="inpe")
            nc.sync.dma_start(inpe_t[:], inpe[:].rearrange(
                "p (m c) -> p m c", m=MT))
            b2_t = persist.tile([128, CE], fp32, tag="b2")
            nc.sync.dma_start(b2_t[:], b2r[:])

            # ---------- scalar ring: mm1 fp8 inputs, then W2 stream -------
            xp_t = persist.tile([128, KP1, 2, B], fp8, tag="xp8", name="xp8")
            nc.scalar.dma_start(xp_t[:], xp8[:])
            w1p_t = []
            for m in range(HM):
                tp = wst.tile([128, KP1, 2, 128], fp8, tag="pblk", bufs=6,
                              name=f"w1pm{m}")
                nc.scalar.dma_start(tp[:], w1pm[m].rearrange(
                    "p (j i c) -> p j i c", i=2, c=128))
                w1p_t.append(tp)

            def load_w2(ch, g):
                w16t = wst.tile([128, NCORES, CC], fp16, tag="wblk", bufs=6,
                                name=f"w16t_{ch}_{g}")
                nc.scalar.dma_start(w16t[:], w216[ch, g].rearrange(
                    "p (s c) -> p s c", c=CC))
                wp8t = wst.tile([128, NCORES, 2, CC], fp8, tag="pblk", bufs=6,
                                name=f"wp8t_{ch}_{g}")
                nc.scalar.dma_start(wp8t[:, :, 0, :], w2p8[ch, g].rearrange(
                    "p (s c) -> p s c", c=CC))
                # fp8 hi slot derived on-chip: e4m3(2 * w16)
                nc.vector.tensor_single_scalar(wp8t[:, :, 1, :], w16t[:],
                                               SC_WH, Alu.mult)
                return w16t, wp8t

            w2b = {(0, 0): load_w2(0, 0), (1, 0): load_w2(1, 0)}

            # ---------- PSUM: per-m accumulator banks ----------
            pm_t = [ps.tile([128, 512], fp32, tag=f"pm{m}", name=f"pm{m}")
                    for m in range(MT)]
            pc_t = [ps.tile([128, 512], fp32, tag=f"pc{m}", name=f"pc{m}")
                    for m in range(MT)]

            # ---------- gather buffers (h16 | hlo8 packed, uint8) ----------
            ag_in = [dram.tile([128, 1536], u8, tag=f"ag_in{m}",
                               name=f"ag_in{m}") for m in range(HM)]
            ag_out = [dram.tile([NCORES, 128, 1536], u8, tag=f"ag_out{m}",
                                name=f"ag_out{m}", addr_space="Shared")
                      for m in range(HM)]

            # ---------- phase 1: mm1 -> h chunk m, split, gather ----------
            for m in range(HM):
                acc = pm_t[m][:]
                for k in range(KT1):
                    nc.tensor.matmul(acc, w1m_t[m][:, k, :],
                                     xt_t[k // 8][:, k % 8, :],
                                     start=(k == 0), stop=(k == KT1 - 1))
                acc2 = pc_t[m][:]
                for j2 in range(KP1):
                    nc.tensor.matmul(acc2, w1p_t[m][:, j2, :, :],
                                     xp_t[:, j2, :, :], perf_mode=DR,
                                     start=(j2 == 0), stop=(j2 == KP1 - 1))
                cds = hwork.tile([128, B], fp32, tag="cds")
                nc.scalar.activation(cds[:], acc2, Act.Copy, scale=DESC)
                zc = hwork.tile([128, B], fp32, tag="zc")
                nc.vector.tensor_tensor(zc[:], cds[:], acc, Alu.add)
                hf = hwork.tile([128, B], fp32, tag="hf")
                nc.vector.tensor_scalar(hf[:], zc[:], b1_t[:, m:m + 1], 0.0,
                                        Alu.add, Alu.max)
                agb = hwork.tile([128, 1536], u8, tag="agb", bufs=1)
                h16v = agb[:, 0:1024].bitcast(fp16)
                nc.scalar.activation(h16v, hf[:], Act.Copy)
                hd = hwork.tile([128, B], fp32, tag="cds", name="hd")
                nc.vector.tensor_tensor(hd[:], hf[:], h16v, Alu.subtract)
                nc.scalar.activation(agb[:, 1024:1536].bitcast(fp8), hd[:],
                                     Act.Copy, scale=SC_H)
                nc.sync.dma_start(ag_in[m][:], agb[:])
                nc.gpsimd.collective_compute(
                    "AllGather", Alu.bypass,
                    replica_groups=[list(range(NCORES))],
                    ins=[ag_in[m].opt()], outs=[ag_out[m].opt()],
                )
                # next W2 block: reuses the w1m/w1p slot this chunk freed
                nxt = [(0, 1), (1, 1), (0, 2), None][m]
                if nxt is not None:
                    w2b[nxt] = load_w2(*nxt)

            # ---------- tokens (under mm1 tail) ----------
            tok_t = persist.tile([128, MT, EP], fp32, tag="tok")
            for m in range(MT):
                tmp = ep.tile([128, EP, V], fp32, tag="tokmul", bufs=1)
                nc.vector.tensor_tensor(
                    tmp[:], inpe_t[:, m, :].rearrange("p (e v) -> p e v", v=V),
                    iota_t.unsqueeze(1).broadcast_to([128, EP, V]), Alu.mult)
                nc.vector.tensor_reduce(tok_t[:, m, :], tmp[:],
                                        axis=AxX, op=Alu.add)

            idx_t = persist.tile([128, MT, EP, 2], fp32, tag="idx")

            # ---------- SBUF accumulators for mm2 (per ch, batch tile) ----
            acc_t = [[persist.tile([128, CC], fp32, tag=f"acc{ch}_{m}",
                                   name=f"acc{ch}_{m}") for m in range(MT)]
                     for ch in range(NCH)]

            def drain(g, ch, m4):
                """psum (main + DESC*corr) for (g, ch, m4) -> acc."""
                a = acc_t[ch][m4]
                if g == 0:
                    nc.scalar.activation(a[:], pc_t[m4][:, :CC],
                                         Act.Copy, scale=DESC)
                    nc.vector.tensor_tensor(a[:], a[:], pm_t[m4][:, :CC],
                                            Alu.add)
                    # fold b2 in here (off the critical tail)
                    nc.vector.tensor_tensor(a[:], a[:],
                                            b2_t[:, ch * CC:(ch + 1) * CC],
                                            Alu.add)
                else:
                    tmp = hwork.tile([128, CC], fp32, tag="dtmp", name="dtmp")
                    nc.scalar.activation(tmp[:], pc_t[m4][:, :CC],
                                         Act.Copy, scale=DESC)
                    nc.vector.tensor_tensor(tmp[:], tmp[:], pm_t[m4][:, :CC],
                                            Alu.add)
                    nc.vector.tensor_tensor(a[:], a[:], tmp[:], Alu.add)

            # ---------- epilogue / flow helpers ----------
            def epilogue(ch, m4):
                # fused final drain (g=3 psum still in banks) + argmax
                netE = ep.tile([128, CC], fp32, tag="netE", name="netE")
                nc.scalar.activation(netE[:], pc_t[m4][:, :CC],
                                     Act.Copy, scale=DESC)
                nc.vector.tensor_tensor(netE[:], netE[:], pm_t[m4][:, :CC],
                                        Alu.add)
                nc.vector.tensor_tensor(netE[:], netE[:], acc_t[ch][m4][:],
                                        Alu.add)
                ng = netE[:].rearrange("p (i s v) -> p (i s) v", s=2, v=V)
                gmax = ep.tile([128, 16], fp32, tag="gmax", name="gmax")
                nc.vector.tensor_reduce(gmax[:], ng, axis=AxX, op=Alu.max)
                eq = ep.tile([128, 16, V], fp32, tag="eq", name="eq")
                nc.vector.tensor_tensor(
                    eq[:], ng,
                    gmax[:].unsqueeze(2).broadcast_to([128, 16, V]),
                    Alu.is_ge)
                nc.vector.tensor_tensor(
                    eq[:], eq[:],
                    cbi_t.unsqueeze(1).broadcast_to([128, 16, V]), Alu.mult)
                tmax = ep.tile([128, 16], fp32, tag="tmax", name="tmax")
                nc.vector.tensor_reduce(tmax[:], eq[:], axis=AxX, op=Alu.max)
                nc.vector.tensor_scalar(
                    idx_t[:, m4, ch * 8:(ch + 1) * 8, :],
                    tmax[:].rearrange("p (i s) -> p i s", s=2),
                    -1.0, BIG, Alu.mult, Alu.add)

            def mod23(dst_tag, src, sh, eng):
                dd = small.tile(sh, fp32, tag=dst_tag + "_d",
                                name=dst_tag + "_d")
                eng.tensor_scalar(dd[:], src, 1.0 / 23.0, -0.49,
                                        Alu.mult, Alu.add)
                qq = small.tile(sh, fp32, tag=dst_tag + "_q",
                                name=dst_tag + "_q")
                eng.tensor_scalar(qq[:], dd[:], MAGIC, MAGIC,
                                        Alu.add, Alu.subtract)
                rr = small.tile(sh, fp32, tag=dst_tag + "_r",
                                name=dst_tag + "_r")
                eng.scalar_tensor_tensor(rr[:], qq[:], -23.0, src,
                                               Alu.mult, Alu.add)
                return rr

            def flow_out(ch, eng):
                cs = slice(ch * 8, (ch + 1) * 8)
                loc = idx_t[:, :, cs, 0]
                scl = idx_t[:, :, cs, 1]
                tks = tok_t[:, :, cs]
                sh = [128, MT, 8]
                u0 = small.tile(sh, fp32, tag="u0", name="u0")
                eng.scalar_tensor_tensor(u0[:], tks, 23.0, loc,
                                               Alu.add, Alu.subtract)
                geu = small.tile(sh, fp32, tag="geu", name="geu")
                eng.tensor_single_scalar(geu[:], u0[:], 23.0, Alu.is_ge)
                u = small.tile(sh, fp32, tag="u", name="u")
                eng.scalar_tensor_tensor(u[:], geu[:], -23.0, u0[:],
                                               Alu.mult, Alu.add)
                # inv(scale) via one-hot LUT: sum_a (scl==a) * invtab[a]
                eqv = ep.tile([128, MT, 8, V], fp32, tag="eqv", name="eqv",
                              bufs=2)
                eng.tensor_tensor(
                    eqv[:],
                    scl.unsqueeze(3).broadcast_to([128, MT, 8, V]),
                    iota_t.unsqueeze(1).unsqueeze(1)
                    .broadcast_to([128, MT, 8, V]), Alu.is_equal)
                eng.tensor_tensor(
                    eqv[:], eqv[:],
                    invt_t.unsqueeze(1).unsqueeze(1)
                    .broadcast_to([128, MT, 8, V]), Alu.mult)
                inv = small.tile(sh, fp32, tag="inv", name="inv")
                # X-axis reduce is DVE-only
                nc.vector.tensor_reduce(inv[:], eqv[:], axis=AxX, op=Alu.add)
                wprod = small.tile(sh, fp32, tag="wprod", name="wprod")
                eng.tensor_tensor(wprod[:], inv[:], u[:], Alu.mult)
                wm = mod23("wm", wprod[:], sh, eng)
                live = small.tile(sh, fp32, tag="live", name="live")
                eng.tensor_single_scalar(live[:], inv[:], 0.5, Alu.is_ge)
                w1pt = small.tile(sh, fp32, tag="w1p", name="w1p")
                eng.tensor_single_scalar(w1pt[:], wm[:], 1.0, Alu.add)
                w2pt = small.tile(sh, fp32, tag="w2p", name="w2p")
                eng.tensor_tensor(w2pt[:], w1pt[:], live[:], Alu.mult)
                wfin = small.tile(sh, fp32, tag="wfin", name="wfin")
                eng.tensor_single_scalar(wfin[:], w2pt[:], -1.0, Alu.add)
                oh = ep.tile([128, MT, 8, V], fp32, tag="oh", name="oh",
                             bufs=2)
                eng.tensor_tensor(
                    oh[:],
                    iota_t.unsqueeze(1).unsqueeze(1)
                    .broadcast_to([128, MT, 8, V]),
                    wfin[:].unsqueeze(3).broadcast_to([128, MT, 8, V]),
                    Alu.is_equal)
                nc.sync.dma_start(
                    oute[:].rearrange("p (m e v) -> p m e v", m=MT, v=V)
                    [:, :, cs, :], oh[:])

            # ---------- phase 2+3: per gathered chunk g, BOTH ch sweeps ---
            # psum accumulates per (g, ch); drained to SBUF accs between
            # chunks so one bank set serves all 8 (g, ch) sweeps.
            for g in range(HM):
                th = persist.tile([128, NCORES, B], fp16, tag="hth", bufs=2,
                                  name=f"hthg{g}")
                pr = persist.tile([128, NCORES, 2, B], fp8, tag="pair",
                                  bufs=2, name=f"pairg{g}")
                src16 = ag_out[g][:, :, 0:1024].bitcast(fp16)
                src8 = ag_out[g][:, :, 1024:1536].bitcast(fp8)
                nc.sync.dma_start(th[:, 0:4, :], src16[0:4].rearrange(
                    "s p c -> p s c"))
                nc.sync.dma_start(pr[:, 0:4, 1, :], src8[0:4].rearrange(
                    "s p c -> p s c"))
                nc.sync.dma_start(th[:, 4:8, :], src16[4:8].rearrange(
                    "s p c -> p s c"))
                nc.sync.dma_start(pr[:, 4:8, 1, :], src8[4:8].rearrange(
                    "s p c -> p s c"))
                # e4m3(h16) into pair slot 0 (vector + gpsimd; scalar is
                # busy issuing W2 triggers and drain copies)
                nc.vector.tensor_copy(pr[:, 0:4, 0, :], th[:, 0:4, :])
                nc.gpsimd.tensor_copy(pr[:, 4:8, 0, :], th[:, 4:8, :])

                for ch in range(NCH):
                    w16t, wp8t = w2b[(ch, g)]
                    last = (g == HM - 1)
                    for m4 in range(MT):
                        for s in range(NCORES):
                            nc.tensor.matmul(
                                pm_t[m4][:, :CC],
                                th[:, s, m4 * 128:(m4 + 1) * 128],
                                w16t[:, s, :],
                                start=(s == 0), stop=(s == NCORES - 1))
                    for m4 in range(MT):
                        for s in range(NCORES):
                            nc.tensor.matmul(
                                pc_t[m4][:, :CC],
                                pr[:, s, :, m4 * 128:(m4 + 1) * 128],
                                wp8t[:, s, :, :], perf_mode=DR,
                                start=(s == 0), stop=(s == NCORES - 1))
                        if last:
                            epilogue(ch, m4)
                            if ch == 0 and m4 == MT - 1:
                                flow_out(0, nc.vector)
                        else:
                            drain(g, ch, m4)
                    # stream remaining W2 blocks as their slots free up
                    if g == 0 and ch == 0:
                        w2b[(1, 2)] = load_w2(1, 2)
                    elif g == 0 and ch == 1:
                        w2b[(0, 3)] = load_w2(0, 3)
                    elif g == 1 and ch == 0:
                        w2b[(1, 3)] = load_w2(1, 3)
                if g == HM - 1:
                    flow_out(1, nc.vector)

    nc.compile()
    return nc


def _e4(a, scale):
    return np.clip(a * scale, -240.0, 240.0).astype(E4M3)


def kernel(inputs, mask, W1, b1, W2, b2):
    from concourse.bass_utils import run_bass_kernel_spmd

    if "nc" not in _cache:
        _cache["nc"] = _build()
    nc = _cache["nc"]

    inputs = np.asarray(inputs, np.float32)
    mask = np.asarray(mask, np.float32)
    W1 = np.asarray(W1, np.float32)
    b1 = np.asarray(b1, np.float32)
    W2 = np.asarray(W2, np.float32)
    b2 = np.asarray(b2, np.float32)

    masked = inputs * mask[None, :, :]
    x_odd = masked[:, 1::2, :].reshape(B, (L // 2) * V)   # [512, 2944]
    xT = np.ascontiguousarray(x_odd.T)                    # [2944, 512]
    xgpad = np.zeros((XG * 8 * 128, B), np.float32)
    xgpad[:KT1 * 128] = xT
    # [g2][p][e*B+b] = x[128*(8*g2+e)+p, b]
    xtg_np = np.ascontiguousarray(
        xgpad.reshape(XG, 8, 128, B).transpose(0, 2, 1, 3)
        .reshape(XG, 128, 8 * B)).astype(FP16)
    xpad = np.zeros((KP1 * 256, B), np.float32)
    xpad[:KT1 * 128] = xT
    xp8_np = np.ascontiguousarray(
        xpad.reshape(KP1, 2, 128, B).transpose(2, 0, 1, 3)).astype(E4M3)

    W1_odd = W1.reshape(L, V, H)[1::2].reshape((L // 2) * V, H)

    in_maps = []
    for k in range(NCORES):
        w1s = W1_odd[:, k * HS:(k + 1) * HS]              # [2944, 512]
        w1hi = w1s.astype(FP16)
        w1lo8 = _e4(w1s - w1hi.astype(np.float32), SC_W).astype(np.float32)
        # [m][p][k*128+c] = w1hi[128k+p, 128m+c]
        w1m_np = np.ascontiguousarray(
            w1hi.reshape(KT1, 128, HM, 128).transpose(2, 1, 0, 3)
            .reshape(HM, 128, KT1 * 128))
        lpad = np.zeros((KP1 * 256, HS), np.float32)
        lpad[:KT1 * 128] = w1lo8
        # [m][p][j2*256 + i*128 + c] = w1lo8[256j2+128i+p, 128m+c]
        w1pm_np = np.ascontiguousarray(
            lpad.reshape(KP1, 2, 128, HM, 128).transpose(3, 2, 0, 1, 4)
            .reshape(HM, 128, KP1 * 256)).astype(E4M3)

        w2sl = W2[:, k * CW:(k + 1) * CW].reshape(H, PS, 2 * V)[:, 0::2, :]
        w2sl = np.ascontiguousarray(w2sl.reshape(H, CE))  # [4096, 736]
        w16 = w2sl.astype(FP16)
        w16f = w16.astype(np.float32)
        wlo8 = _e4(w2sl - w16f, SC_W).astype(np.float32)
        # [ch][g][p][s*CC+c] = w16[128*(4s+g)+p, 368ch+c]
        wr = w16.reshape(NCORES, HM, 128, NCH, CC)
        w216_np = np.ascontiguousarray(
            wr.transpose(3, 1, 2, 0, 4).reshape(NCH, HM, 128, 8 * CC))
        wl = wlo8.reshape(NCORES, HM, 128, NCH, CC)
        w2p8_np = np.ascontiguousarray(
            wl.transpose(3, 1, 2, 0, 4)
            .reshape(NCH, HM, 128, 8 * CC)).astype(E4M3)

        b2s = b2[k * CW:(k + 1) * CW].reshape(PS, 2 * V)[0::2].reshape(CE)
        cols = slice(32 * k, 32 * k + 32, 2)
        inpe_np = inputs[:, cols, :].reshape(MT, 128, EP * V) \
            .transpose(1, 0, 2).reshape(128, MT * EP * V)
        in_maps.append({
            "xtg": xtg_np,
            "xp8": xp8_np,
            "w1m": w1m_np,
            "w1pm": w1pm_np,
            "b1s": np.ascontiguousarray(
                b1[k * HS:(k + 1) * HS].reshape(HM, 128).T),
            "w216": w216_np,
            "w2p8": w2p8_np,
            "b2r": np.ascontiguousarray(np.broadcast_to(b2s, (128, CE))),
            "inpe": np.ascontiguousarray(inpe_np),
        })

    res = run_bass_kernel_spmd(nc, in_maps, core_ids=list(range(NCORES)))
    _cache["last_result"] = res

    out = np.empty((B, L, V), np.float32)
    out[:, 1::2, :] = masked[:, 1::2, :]
    for k in range(NCORES):
        oe = res.results[k]["oute"].reshape(128, MT, EP, V) \
            .transpose(1, 0, 2, 3).reshape(B, EP, V)
        out[:, 32 * k:32 * k + 32:2, :] = oe
    return out


# revision 17
# speedup vs baseline: 1.1007x; 1.1007x over previous
"""DiscreteBipartiteFlow forward on 8 Trainium2 NeuronCores.

Math (forward pass only):
  masked = mask * inputs                      (mask = 1 at odd l, 0 at even l)
  h   = relu(masked.reshape(B, L*V) @ W1 + b1)
  net = (h @ W2 + b2).reshape(B, L, 2V)
  loc, scale = argmax one-hots of net[..., :V], net[..., V:]
  out[odd l]  = inputs
  out[even l] = onehot((inv(scale) * ((tok - loc) mod V)) mod V), or 0 if scale==0

Sharding (8 cores): mm1 tensor-parallel over hidden; all-gather h; mm2
tensor-parallel over output columns (only the 736 even-position columns).

Precision: fp16 main passes + ONE DoubleRow dual-fp8 correction pass
(identical numerics to the validated baseline: 4 argmax flips, rel_err
0.0074, gate 2e-2 ~ 26 flips).

Schedule (v2): the runtime comm BARRIER (~50us) runs concurrently with
the input DMAs + mm1; the h AllGathers are the first collectives (no
warm-up dummy).  W2 streams through the SAME rotating weight buffers
mm1's weights used (tag-shared tile pool, 5 bufs each), issued on the
scalar HWDGE ring ahead of all AG-gated traffic, so ~7 of 8 blocks land
before the gather window opens.  The gathered-h unpack rides the sync
ring (idle after the input loads).  w2 fp8 hi slots are derived on-chip
from the fp16 blocks (saves 3MB of hot-window HBM).  mm2 runs per
column-half; the last block group and all of sweep 2 run batch-tile-
major with per-(half, batch-tile) epilogues so argmax/flow hide under
the remaining matmuls; the flow inverse uses a one-hot LUT reduce
instead of a serial mod-exponentiation chain.
"""

import numpy as np
import ml_dtypes

B, L, V = 512, 256, 23
H = 4096
NCORES = 8
HS = H // NCORES          # 512  hidden shard
HM = HS // 128            # 4    local hidden tiles
PS = L // NCORES          # 32   positions per core
EP = PS // 2              # 16   even positions per core
CW = PS * 2 * V           # 1472 net columns per core (incl. unused odd)
CE = EP * 2 * V           # 736  even-position net columns
NCH = 2                   # column-half sweeps for mm2
CC = CE // NCH            # 368  columns per sweep
KT1 = (L // 2) * V // 128  # 23 contraction tiles for mm1
KP1 = 12                  # mm1 pair tiles (2944 rows -> 1472 pairs, padded)
XG = 3                    # xt DMA groups of 8 k-tiles (8+8+7)
KT2 = H // 128            # 32 contraction tiles for mm2
MT = B // 128             # 4 batch tiles

SC_W = 2.0**13            # w_lo / w1_lo quantization scale
SC_H = 2.0**12            # h_lo quantization scale
SC_WH = 2.0               # w_hi quantization scale (12 + 1 = 13)
DESC = 2.0**-13           # corr descale

BIG = 64.0
MAGIC = 12582912.0        # 1.5 * 2^23: float32 round-to-int domain
FP16 = np.float16
E4M3 = ml_dtypes.float8_e4m3   # TRN FP8_EXP4: max +-240, matches in-range

_cache = {}


def _inv_table():
    return np.array([0] + [pow(a, -1, V) for a in range(1, V)],
                    dtype=np.float32)


def _build():
    import concourse.mybir as mybir
    import concourse.tile as tile
    from concourse import bacc

    fp32 = mybir.dt.float32
    fp16 = mybir.dt.float16
    fp8 = mybir.dt.float8e4
    u8 = mybir.dt.uint8
    Alu = mybir.AluOpType
    Act = mybir.ActivationFunctionType
    DR = mybir.MatmulPerfMode.DoubleRow
    AxX = mybir.AxisListType.X

    nc = bacc.Bacc("TRN2", target_bir_lowering=False, debug=False,
                   num_devices=NCORES)

    # ---- per-core inputs ----
    xtg = nc.dram_tensor("xtg", [XG, 128, 8 * B], fp16, kind="ExternalInput")
    xp8 = nc.dram_tensor("xp8", [128, KP1, 2, B], fp8, kind="ExternalInput")
    w1m = nc.dram_tensor("w1m", [HM, 128, KT1 * 128], fp16,
                         kind="ExternalInput")
    w1pm = nc.dram_tensor("w1pm", [HM, 128, KP1 * 256], fp8,
                          kind="ExternalInput")
    b1s = nc.dram_tensor("b1s", [128, HM], fp32, kind="ExternalInput")
    w216 = nc.dram_tensor("w216", [NCH, HM, 128, 8 * CC], fp16,
                          kind="ExternalInput")
    w2p8 = nc.dram_tensor("w2p8", [NCH, HM, 128, 8 * CC], fp8,
                          kind="ExternalInput")
    b2r = nc.dram_tensor("b2r", [128, CE], fp32, kind="ExternalInput")
    inpe = nc.dram_tensor("inpe", [128, MT * EP * V], fp32,
                          kind="ExternalInput")
    oute = nc.dram_tensor("oute", [128, MT * EP * V], fp32,
                          kind="ExternalOutput")

    # ---- constants: iota | BIG-iota | inv_table ----
    iota_np = np.arange(V, dtype=np.float32)[None, :].repeat(128, 0)
    cc_np = np.concatenate(
        [iota_np, BIG - iota_np, _inv_table()[None, :].repeat(128, 0)], axis=1)
    c_cc = nc.inline_tensor(np.ascontiguousarray(cc_np), name="c_cc")

    with tile.TileContext(nc) as tc:
        with (
            tc.tile_pool(name="persist", bufs=1) as persist,
            tc.tile_pool(name="wst", bufs=1) as wst,
            tc.tile_pool(name="hwork", bufs=2) as hwork,
            tc.tile_pool(name="ep", bufs=2) as ep,
            tc.tile_pool(name="small", bufs=1) as small,
            tc.tile_pool(name="ps", bufs=1, space="PSUM") as ps,
            tc.tile_pool(name="dram", bufs=1, space="DRAM") as dram,
        ):
            # ---------- dummy collective: absorbs first-collective setup --
            warm_in = dram.tile([1, 16], fp32, tag="warm_in")
            warm_out = dram.tile([NCORES, 16], fp32, tag="warm_out",
                                 addr_space="Shared")
            nc.gpsimd.collective_compute(
                "AllGather", Alu.bypass,
                replica_groups=[list(range(NCORES))],
                ins=[warm_in.opt()], outs=[warm_out.opt()],
            )

            # ---------- bulk loads: sync ring (mm1-critical first) --------
            cc_t = persist.tile([128, 3 * V], fp32, tag="ccst")
            nc.sync.dma_start(cc_t[:], c_cc[:])
            iota_t = cc_t[:, :V]
            cbi_t = cc_t[:, V:2 * V]
            invt_t = cc_t[:, 2 * V:]
            b1_t = persist.tile([128, HM], fp32, tag="b1")
            nc.sync.dma_start(b1_t[:], b1s[:])
            xt_t = []
            for g2 in range(XG):
                ng2 = 8 if g2 < XG - 1 else KT1 - 8 * (XG - 1)
                t = persist.tile([128, ng2, B], fp16, tag=f"xtg{g2}",
                                 name=f"xtg{g2}")
                nc.sync.dma_start(t[:], xtg[g2][:, :ng2 * B].rearrange(
                    "p (e b) -> p e b", b=B))
                xt_t.append(t)
            w1m_t = []
            for m in range(HM):
                tw = wst.tile([128, KT1, 128], fp16, tag="wblk", bufs=6,
                              name=f"w1m{m}")
                nc.sync.dma_start(tw[:], w1m[m].rearrange(
                    "p (k c) -> p k c", c=128))
                w1m_t.append(tw)
            inpe_t = persist.tile([128, MT, EP * V], fp32, tag="inpe")
            nc.sync.dma_start(inpe_t[:], inpe[:].rearrange(
                "p (m c) -> p m c", m=MT))
            b2_t = persist.tile([128, CE], fp32, tag="b2")
            nc.sync.dma_start(b2_t[:], b2r[:])

            # ---------- scalar ring: mm1 fp8 inputs, then W2 stream -------
            xp_t = persist.tile([128, KP1, 2, B], fp8, tag="xp8", name="xp8")
            nc.scalar.dma_start(xp_t[:], xp8[:])
            w1p_t = []
            for m in range(HM):
                tp = wst.tile([128, KP1, 2, 128], fp8, tag="pblk", bufs=6,
                              name=f"w1pm{m}")
                nc.scalar.dma_start(tp[:], w1pm[m].rearrange(
                    "p (j i c) -> p j i c", i=2, c=128))
                w1p_t.append(tp)

            def load_w2(ch, g):
                w16t = wst.tile([128, NCORES, CC], fp16, tag="wblk", bufs=6,
                                name=f"w16t_{ch}_{g}")
                nc.scalar.dma_start(w16t[:], w216[ch, g].rearrange(
                    "p (s c) -> p s c", c=CC))
                wp8t = wst.tile([128, NCORES, 2, CC], fp8, tag="pblk", bufs=6,
                                name=f"wp8t_{ch}_{g}")
                nc.scalar.dma_start(wp8t[:, :, 0, :], w2p8[ch, g].rearrange(
                    "p (s c) -> p s c", c=CC))
                # fp8 hi slot derived on-chip: e4m3(2 * w16)
                nc.vector.tensor_single_scalar(wp8t[:, :, 1, :], w16t[:],
                                               SC_WH, Alu.mult)
                return w16t, wp8t

            w2b = {(0, 0): load_w2(0, 0), (1, 0): load_w2(1, 0)}

            # ---------- PSUM: per-m accumulator banks ----------
            pm_t = [ps.tile([128, 512], fp32, tag=f"pm{m}", name=f"pm{m}")
                    for m in range(MT)]
            pc_t = [ps.tile([128, 512], fp32, tag=f"pc{m}", name=f"pc{m}")
                    for m in range(MT)]

            # ---------- gather buffers (h16 | hlo8 packed, uint8) ----------
            ag_in = [dram.tile([128, 1536], u8, tag=f"ag_in{m}",
                               name=f"ag_in{m}") for m in range(HM)]
            ag_out = [dram.tile([NCORES, 128, 1536], u8, tag=f"ag_out{m}",
                                name=f"ag_out{m}", addr_space="Shared")
                      for m in range(HM)]

            # ---------- phase 1: mm1 -> h chunk m, split, gather ----------
            for m in range(HM):
                acc = pm_t[m][:]
                for k in range(KT1):
                    nc.tensor.matmul(acc, w1m_t[m][:, k, :],
                                     xt_t[k // 8][:, k % 8, :],
                                     start=(k == 0), stop=(k == KT1 - 1))
                acc2 = pc_t[m][:]
                for j2 in range(KP1):
                    nc.tensor.matmul(acc2, w1p_t[m][:, j2, :, :],
                                     xp_t[:, j2, :, :], perf_mode=DR,
                                     start=(j2 == 0), stop=(j2 == KP1 - 1))
                cds = hwork.tile([128, B], fp32, tag="cds")
                nc.scalar.activation(cds[:], acc2, Act.Copy, scale=DESC)
                zc = hwork.tile([128, B], fp32, tag="zc")
                nc.vector.tensor_tensor(zc[:], cds[:], acc, Alu.add)
                hf = hwork.tile([128, B], fp32, tag="hf")
                nc.vector.tensor_scalar(hf[:], zc[:], b1_t[:, m:m + 1], 0.0,
                                        Alu.add, Alu.max)
                agb = hwork.tile([128, 1536], u8, tag="agb", bufs=1)
                h16v = agb[:, 0:1024].bitcast(fp16)
                nc.scalar.activation(h16v, hf[:], Act.Copy)
                hd = hwork.tile([128, B], fp32, tag="cds", name="hd")
                nc.vector.tensor_tensor(hd[:], hf[:], h16v, Alu.subtract)
                nc.scalar.activation(agb[:, 1024:1536].bitcast(fp8), hd[:],
                                     Act.Copy, scale=SC_H)
                nc.sync.dma_start(ag_in[m][:], agb[:])
                nc.gpsimd.collective_compute(
                    "AllGather", Alu.bypass,
                    replica_groups=[list(range(NCORES))],
                    ins=[ag_in[m].opt()], outs=[ag_out[m].opt()],
                )
                # next W2 block: reuses the w1m/w1p slot this chunk freed
                nxt = [(0, 1), (1, 1), (0, 2), None][m]
                if nxt is not None:
                    w2b[nxt] = load_w2(*nxt)

            # ---------- tokens (under mm1 tail) ----------
            tok_t = persist.tile([128, MT, EP], fp32, tag="tok")
            for m in range(MT):
                tmp = ep.tile([128, EP, V], fp32, tag="tokmul", bufs=1)
                nc.vector.tensor_tensor(
                    tmp[:], inpe_t[:, m, :].rearrange("p (e v) -> p e v", v=V),
                    iota_t.unsqueeze(1).broadcast_to([128, EP, V]), Alu.mult)
                nc.vector.tensor_reduce(tok_t[:, m, :], tmp[:],
                                        axis=AxX, op=Alu.add)

            idx_t = persist.tile([128, MT, EP, 2], fp32, tag="idx")

            # ---------- SBUF accumulators for mm2 (per ch, batch tile) ----
            acc_t = [[persist.tile([128, CC], fp32, tag=f"acc{ch}_{m}",
                                   name=f"acc{ch}_{m}") for m in range(MT)]
                     for ch in range(NCH)]

            def drain(g, ch, m4):
                """psum (main + DESC*corr) for (g, ch, m4) -> acc."""
                a = acc_t[ch][m4]
                if g == 0:
                    nc.scalar.activation(a[:], pc_t[m4][:, :CC],
                                         Act.Copy, scale=DESC)
                    nc.vector.tensor_tensor(a[:], a[:], pm_t[m4][:, :CC],
                                            Alu.add)
                    # fold b2 in here (off the critical tail)
                    nc.vector.tensor_tensor(a[:], a[:],
                                            b2_t[:, ch * CC:(ch + 1) * CC],
                                            Alu.add)
                else:
                    tmp = hwork.tile([128, CC], fp32, tag="dtmp", name="dtmp")
                    nc.scalar.activation(tmp[:], pc_t[m4][:, :CC],
                                         Act.Copy, scale=DESC)
                    nc.vector.tensor_tensor(tmp[:], tmp[:], pm_t[m4][:, :CC],
                                            Alu.add)
                    nc.vector.tensor_tensor(a[:], a[:], tmp[:], Alu.add)

            # ---------- epilogue / flow helpers ----------
            def epilogue(ch, m4):
                # fused final drain (g=3 psum still in banks) + argmax
                netE = ep.tile([128, CC], fp32, tag="netE", name="netE")
                nc.scalar.activation(netE[:], pc_t[m4][:, :CC],
                                     Act.Copy, scale=DESC)
                nc.vector.tensor_tensor(netE[:], netE[:], pm_t[m4][:, :CC],
                                        Alu.add)
                nc.vector.tensor_tensor(netE[:], netE[:], acc_t[ch][m4][:],
                                        Alu.add)
                ng = netE[:].rearrange("p (i s v) -> p (i s) v", s=2, v=V)
                gmax = ep.tile([128, 16], fp32, tag="gmax", name="gmax")
                nc.vector.tensor_reduce(gmax[:], ng, axis=AxX, op=Alu.max)
                eq = ep.tile([128, 16, V], fp32, tag="eq", name="eq")
                nc.vector.tensor_tensor(
                    eq[:], ng,
                    gmax[:].unsqueeze(2).broadcast_to([128, 16, V]),
                    Alu.is_ge)
                nc.vector.tensor_tensor(
                    eq[:], eq[:],
                    cbi_t.unsqueeze(1).broadcast_to([128, 16, V]), Alu.mult)
                tmax = ep.tile([128, 16], fp32, tag="tmax", name="tmax")
                nc.vector.tensor_reduce(tmax[:], eq[:], axis=AxX, op=Alu.max)
                nc.vector.tensor_scalar(
                    idx_t[:, m4, ch * 8:(ch + 1) * 8, :],
                    tmax[:].rearrange("p (i s) -> p i s", s=2),
                    -1.0, BIG, Alu.mult, Alu.add)

            def mod23(dst_tag, src, sh, eng):
                dd = small.tile(sh, fp32, tag=dst_tag + "_d",
                                name=dst_tag + "_d")
                eng.tensor_scalar(dd[:], src, 1.0 / 23.0, -0.49,
                                        Alu.mult, Alu.add)
                qq = small.tile(sh, fp32, tag=dst_tag + "_q",
                                name=dst_tag + "_q")
                eng.tensor_scalar(qq[:], dd[:], MAGIC, MAGIC,
                                        Alu.add, Alu.subtract)
                rr = small.tile(sh, fp32, tag=dst_tag + "_r",
                                name=dst_tag + "_r")
                eng.scalar_tensor_tensor(rr[:], qq[:], -23.0, src,
                                               Alu.mult, Alu.add)
                return rr

            def flow_out(ch, eng):
                cs = slice(ch * 8, (ch + 1) * 8)
                loc = idx_t[:, :, cs, 0]
                scl = idx_t[:, :, cs, 1]
                tks = tok_t[:, :, cs]
                sh = [128, MT, 8]
                u0 = small.tile(sh, fp32, tag="u0", name="u0")
                eng.scalar_tensor_tensor(u0[:], tks, 23.0, loc,
                                               Alu.add, Alu.subtract)
                geu = small.tile(sh, fp32, tag="geu", name="geu")
                eng.tensor_single_scalar(geu[:], u0[:], 23.0, Alu.is_ge)
                u = small.tile(sh, fp32, tag="u", name="u")
                eng.scalar_tensor_tensor(u[:], geu[:], -23.0, u0[:],
                                               Alu.mult, Alu.add)
                # inv(scale) via one-hot LUT: sum_a (scl==a) * invtab[a]
                eqv = ep.tile([128, MT, 8, V], fp32, tag="eqv", name="eqv",
                              bufs=2)
                eng.tensor_tensor(
                    eqv[:],
                    scl.unsqueeze(3).broadcast_to([128, MT, 8, V]),
                    iota_t.unsqueeze(1).unsqueeze(1)
                    .broadcast_to([128, MT, 8, V]), Alu.is_equal)
                eng.tensor_tensor(
                    eqv[:], eqv[:],
                    invt_t.unsqueeze(1).unsqueeze(1)
                    .broadcast_to([128, MT, 8, V]), Alu.mult)
                inv = small.tile(sh, fp32, tag="inv", name="inv")
                # X-axis reduce is DVE-only
                nc.vector.tensor_reduce(inv[:], eqv[:], axis=AxX, op=Alu.add)
                wprod = small.tile(sh, fp32, tag="wprod", name="wprod")
                eng.tensor_tensor(wprod[:], inv[:], u[:], Alu.mult)
                wm = mod23("wm", wprod[:], sh, eng)
                live = small.tile(sh, fp32, tag="live", name="live")
                eng.tensor_single_scalar(live[:], inv[:], 0.5, Alu.is_ge)
                w1pt = small.tile(sh, fp32, tag="w1p", name="w1p")
                eng.tensor_single_scalar(w1pt[:], wm[:], 1.0, Alu.add)
                w2pt = small.tile(sh, fp32, tag="w2p", name="w2p")
                eng.tensor_tensor(w2pt[:], w1pt[:], live[:], Alu.mult)
                wfin = small.tile(sh, fp32, tag="wfin", name="wfin")
                eng.tensor_single_scalar(wfin[:], w2pt[:], -1.0, Alu.add)
                oh = ep.tile([128, MT, 8, V], fp32, tag="oh", name="oh",
                             bufs=2)
                eng.tensor_tensor(
                    oh[:],
                    iota_t.unsqueeze(1).unsqueeze(1)
                    .broadcast_to([128, MT, 8, V]),
                    wfin[:].unsqueeze(3).broadcast_to([128, MT, 8, V]),
                    Alu.is_equal)
                nc.sync.dma_start(
                    oute[:].rearrange("p (m e v) -> p m e v", m=MT, v=V)
                    [:, :, cs, :], oh[:])

            # ---------- phase 2+3: per gathered chunk g, BOTH ch sweeps ---
            # psum accumulates per (g, ch); drained to SBUF accs between
            # chunks so one bank set serves all 8 (g, ch) sweeps.
            for g in range(HM):
                th = persist.tile([128, NCORES, B], fp16, tag="hth", bufs=2,
                                  name=f"hthg{g}")
                pr = persist.tile([128, NCORES, 2, B], fp8, tag="pair",
                                  bufs=2, name=f"pairg{g}")
                src16 = ag_out[g][:, :, 0:1024].bitcast(fp16)
                src8 = ag_out[g][:, :, 1024:1536].bitcast(fp8)
                nc.sync.dma_start(th[:, 0:4, :], src16[0:4].rearrange(
                    "s p c -> p s c"))
                nc.sync.dma_start(pr[:, 0:4, 1, :], src8[0:4].rearrange(
                    "s p c -> p s c"))
                nc.sync.dma_start(th[:, 4:8, :], src16[4:8].rearrange(
                    "s p c -> p s c"))
                nc.sync.dma_start(pr[:, 4:8, 1, :], src8[4:8].rearrange(
                    "s p c -> p s c"))
                # e4m3(h16) into pair slot 0 (vector + gpsimd; scalar is
                # busy issuing W2 triggers and drain copies).  For the last
                # chunk gpsimd takes both halves: vector starts epilogues.
                if g < HM - 1:
                    nc.vector.tensor_copy(pr[:, 0:4, 0, :], th[:, 0:4, :])
                    nc.gpsimd.tensor_copy(pr[:, 4:8, 0, :], th[:, 4:8, :])
                else:
                    nc.gpsimd.tensor_copy(pr[:, 0:4, 0, :], th[:, 0:4, :])
                    nc.gpsimd.tensor_copy(pr[:, 4:8, 0, :], th[:, 4:8, :])

                for ch in range(NCH):
                    w16t, wp8t = w2b[(ch, g)]
                    last = (g == HM - 1)
                    for m4 in range(MT):
                        for s in range(NCORES):
                            nc.tensor.matmul(
                                pm_t[m4][:, :CC],
                                th[:, s, m4 * 128:(m4 + 1) * 128],
                                w16t[:, s, :],
                                start=(s == 0), stop=(s == NCORES - 1))
                    for m4 in range(MT):
                        for s in range(NCORES):
                            nc.tensor.matmul(
                                pc_t[m4][:, :CC],
                                pr[:, s, :, m4 * 128:(m4 + 1) * 128],
                                wp8t[:, s, :, :], perf_mode=DR,
                                start=(s == 0), stop=(s == NCORES - 1))
                        if last:
                            epilogue(ch, m4)
                            if ch == 0 and m4 == MT - 1:
                                flow_out(0, nc.vector)
                        else:
                            drain(g, ch, m4)
                    # stream remaining W2 blocks as their slots free up
                    if g == 0 and ch == 0:
                        w2b[(1, 2)] = load_w2(1, 2)
                    elif g == 0 and ch == 1:
                        w2b[(0, 3)] = load_w2(0, 3)
                    elif g == 1 and ch == 0:
                        w2b[(1, 3)] = load_w2(1, 3)
                if g == HM - 1:
                    flow_out(1, nc.vector)

    nc.compile()
    return nc


def _e4(a, scale):
    return np.clip(a * scale, -240.0, 240.0).astype(E4M3)


def kernel(inputs, mask, W1, b1, W2, b2):
    from concourse.bass_utils import run_bass_kernel_spmd

    if "nc" not in _cache:
        _cache["nc"] = _build()
    nc = _cache["nc"]

    inputs = np.asarray(inputs, np.float32)
    mask = np.asarray(mask, np.float32)
    W1 = np.asarray(W1, np.float32)
    b1 = np.asarray(b1, np.float32)
    W2 = np.asarray(W2, np.float32)
    b2 = np.asarray(b2, np.float32)

    masked = inputs * mask[None, :, :]
    x_odd = masked[:, 1::2, :].reshape(B, (L // 2) * V)   # [512, 2944]
    xT = np.ascontiguousarray(x_odd.T)                    # [2944, 512]
    xgpad = np.zeros((XG * 8 * 128, B), np.float32)
    xgpad[:KT1 * 128] = xT
    # [g2][p][e*B+b] = x[128*(8*g2+e)+p, b]
    xtg_np = np.ascontiguousarray(
        xgpad.reshape(XG, 8, 128, B).transpose(0, 2, 1, 3)
        .reshape(XG, 128, 8 * B)).astype(FP16)
    xpad = np.zeros((KP1 * 256, B), np.float32)
    xpad[:KT1 * 128] = xT
    xp8_np = np.ascontiguousarray(
        xpad.reshape(KP1, 2, 128, B).transpose(2, 0, 1, 3)).astype(E4M3)

    W1_odd = W1.reshape(L, V, H)[1::2].reshape((L // 2) * V, H)

    in_maps = []
    for k in range(NCORES):
        w1s = W1_odd[:, k * HS:(k + 1) * HS]              # [2944, 512]
        w1hi = w1s.astype(FP16)
        w1lo8 = _e4(w1s - w1hi.astype(np.float32), SC_W).astype(np.float32)
        # [m][p][k*128+c] = w1hi[128k+p, 128m+c]
        w1m_np = np.ascontiguousarray(
            w1hi.reshape(KT1, 128, HM, 128).transpose(2, 1, 0, 3)
            .reshape(HM, 128, KT1 * 128))
        lpad = np.zeros((KP1 * 256, HS), np.float32)
        lpad[:KT1 * 128] = w1lo8
        # [m][p][j2*256 + i*128 + c] = w1lo8[256j2+128i+p, 128m+c]
        w1pm_np = np.ascontiguousarray(
            lpad.reshape(KP1, 2, 128, HM, 128).transpose(3, 2, 0, 1, 4)
            .reshape(HM, 128, KP1 * 256)).astype(E4M3)

        w2sl = W2[:, k * CW:(k + 1) * CW].reshape(H, PS, 2 * V)[:, 0::2, :]
        w2sl = np.ascontiguousarray(w2sl.reshape(H, CE))  # [4096, 736]
        w16 = w2sl.astype(FP16)
        w16f = w16.astype(np.float32)
        wlo8 = _e4(w2sl - w16f, SC_W).astype(np.float32)
        # [ch][g][p][s*CC+c] = w16[128*(4s+g)+p, 368ch+c]
        wr = w16.reshape(NCORES, HM, 128, NCH, CC)
        w216_np = np.ascontiguousarray(
            wr.transpose(3, 1, 2, 0, 4).reshape(NCH, HM, 128, 8 * CC))
        wl = wlo8.reshape(NCORES, HM, 128, NCH, CC)
        w2p8_np = np.ascontiguousarray(
            wl.transpose(3, 1, 2, 0, 4)
            .reshape(NCH, HM, 128, 8 * CC)).astype(E4M3)

        b2s = b2[k * CW:(k + 1) * CW].reshape(PS, 2 * V)[0::2].reshape(CE)
        cols = slice(32 * k, 32 * k + 32, 2)
        inpe_np = inputs[:, cols, :].reshape(MT, 128, EP * V) \
            .transpose(1, 0, 2).reshape(128, MT * EP * V)
        in_maps.append({
            "xtg": xtg_np,
            "xp8": xp8_np,
            "w1m": w1m_np,
            "w1pm": w1pm_np,
            "b1s": np.ascontiguousarray(
                b1[k * HS:(k + 1) * HS].reshape(HM, 128).T),
            "w216": w216_np,
            "w2p8": w2p8_np,
            "b2r": np.ascontiguousarray(np.broadcast_to(b2s, (128, CE))),
            "inpe": np.ascontiguousarray(inpe_np),
        })

    res = run_bass_kernel_spmd(nc, in_maps, core_ids=list(range(NCORES)))
    _cache["last_result"] = res

    out = np.empty((B, L, V), np.float32)
    out[:, 1::2, :] = masked[:, 1::2, :]
    for k in range(NCORES):
        oe = res.results[k]["oute"].reshape(128, MT, EP, V) \
            .transpose(1, 0, 2, 3).reshape(B, EP, V)
        out[:, 32 * k:32 * k + 32:2, :] = oe
    return out
